# revision 1
# baseline (speedup 1.0000x reference)
"""SogCLR loss kernel for 8 Trainium2 NeuronCores.

Math restructure: with B=8192, D=256, T=temperature,
  sim = I @ T^T, diag_i = I_i . T_i, E = exp(sim/T), F = E * sim.
All four needed reductions are plain sums of E and F:
  R_i = sum_j E_ij   (row sums)     P_i = sum_j F_ij
  C_j = sum_i E_ij   (col sums)     Q_j = sum_i F_ij
Then with u_i = exp(-diag_i/T):
  A0_i = u_i R_i                  (= sum_j exp(idt_ij), shift-free)
  N0_i = u_i (P_i - diag_i R_i)/T (= sum_j exp(idt_ij) idt_ij)
  K_i  = (1-g) s_old_i e^{old_b_i} + g A0_i/(B-1)
  image_loss_i = T N0_i / (K_i + eps e^{b_i}) / (B-1)
The exact running max b_i only enters through the eps=1e-10 term;
substituting e^{b_i} ~ A0_i changes the loss by < 1e-5 relative.
Text side is symmetric with C, Q, b_T, s_T.

Device pipeline per core (row shard of 1024):
  - features quantized to fp8e4 (scaled x512 per side), sim tiles
    [128 x 1024] via fp8 DoubleRow matmuls (K=256 in one instruction),
  - exp on ScalarE with fused row-sum accumulate (E' = exp(sim/T - c),
    c = ln4; the uniform factor cancels on host),
  - F' = sim * E' + row-sum accumulate via one DVE scalar_tensor_tensor
    (the DVE runs nothing else in steady state - it is the wall),
  - col sums of E'/F' via bf16 ones-matmuls, 4 concurrent tile_position
    streams per PSUM bank, accumulated across the 8 row stripes.
Row accumulators (racc/pacc) go to DRAM raw; host does the final
8-chunk reduction and all O(B) math in float64.
"""

import os
import sys

import numpy as np

sys.path.insert(0, "/opt/trn_rl_repo")

TEMP = 0.07
GAMMA = 0.1
EPS = 1e-10
B = 8192
D = 256
NCORES = 8
SHARD = B // NCORES          # 1024 rows per core
PDIM = 128
NSTRIPE = SHARD // PDIM      # 8
KCH = D // PDIM              # 2 contraction chunks of 128
FSCALE = 512.0               # per-side fp8 feature scale
SIMSCALE = FSCALE * FSCALE   # sim is scaled by this in PSUM
CSHIFT = float(np.log(4.0))  # exp bias: E' = exp(sim/T - CSHIFT)

_prog = None
last_result = None           # BassKernelResults of the most recent run
_hook_installed = False


def _install_ntff_hook():
    """Register the axon NTFF profile hook that the container boot skipped
    (its antenv stub lacks axon_hooks).  Lets run_bass_kernel_spmd(trace=True)
    return exec_time_ns + a perfetto trace."""
    global _hook_installed
    if _hook_installed:
        return
    import types

    import antenv
    from trn_agent_boot.trn_boot import _ntff_profile_via_ctypes

    mod = types.ModuleType("antenv.axon_hooks")
    holder = {}
    mod.set_axon_ntff_profile_hook = lambda h: holder.__setitem__("h", h)
    mod.get_axon_ntff_profile_hook = lambda: holder.get("h")
    antenv.axon_hooks = mod
    sys.modules["antenv.axon_hooks"] = mod
    mod.set_axon_ntff_profile_hook(
        _ntff_profile_via_ctypes("/opt/axon/libaxon_pjrt.so")
    )
    _hook_installed = True


def _build_program():
    import concourse.tile as tile
    from concourse import bacc, mybir

    f32 = mybir.dt.float32
    bf16 = mybir.dt.bfloat16
    fp8 = mybir.dt.float8e4
    AF = mybir.ActivationFunctionType
    ALU = mybir.AluOpType
    DR = mybir.MatmulPerfMode.DoubleRow

    nc = bacc.Bacc(
        "TRN2", target_bir_lowering=False, debug=False, num_devices=NCORES
    )

    # [p, c, i] holds I^T[c*128+p, i] * FSCALE for this core's 1024 rows
    it_dram = nc.dram_tensor(
        "it_shard", [PDIM, KCH, SHARD], fp8, kind="ExternalInput"
    ).ap()
    # [p, c, j] holds T^T[c*128+p, j] * FSCALE, all 8192 columns
    tt_dram = nc.dram_tensor(
        "tt_full", [PDIM, KCH, B], fp8, kind="ExternalInput"
    ).ap()
    # raw row accumulators: racc/pacc [128, NSTRIPE*NBSLAB]; host reduces
    r_dram = nc.dram_tensor("r_out", [PDIM, 64], f32, kind="ExternalOutput").ap()
    p_dram = nc.dram_tensor("p_out", [PDIM, 64], f32, kind="ExternalOutput").ap()
    # [row, bslab, x]: row 0/1 = colsum(E) even/odd 512-half, 2/3 = colsum(F)
    cq_dram = nc.dram_tensor(
        "cq_out", [4, B // 1024, 512], f32, kind="ExternalOutput"
    ).ap()

    BSLAB = 1024                 # column slab = one sim tile = 2 PSUM banks
    NBSLAB = B // BSLAB          # 8

    with tile.TileContext(nc) as tc:
        with (
            tc.tile_pool(name="singles", bufs=1) as singles,
            tc.tile_pool(name="epool", bufs=5) as epool,
            tc.tile_pool(name="fpool", bufs=5) as fpool,
            tc.tile_pool(name="dpool", bufs=2) as dpool,
            tc.tile_pool(name="psim", bufs=3, space="PSUM") as psim,
            tc.tile_pool(name="pcol", bufs=2, space="PSUM") as pcol,
        ):
            tt_sb = singles.tile([PDIM, KCH, B], fp8)
            it_sb = singles.tile([PDIM, KCH, SHARD], fp8)
            ones_sb = singles.tile([PDIM, 1], bf16)
            bias_sb = singles.tile([PDIM, 1], f32)
            warm_sb = singles.tile([PDIM, 16], bf16)
            warm2_sb = singles.tile([PDIM, 512], bf16)
            racc = singles.tile([PDIM, NSTRIPE * NBSLAB], f32)
            pacc = singles.tile([PDIM, NSTRIPE * NBSLAB], f32)

            # input DMAs: it rides the idle GPSIMD software-DGE queue so its
            # descriptor generation overlaps the SP queue's tt issues; the
            # first tt chunk is split so the first 512-wide matmul unblocks
            # as early as possible
            nc.gpsimd.dma_start(out=it_sb, in_=it_dram)
            nc.sync.dma_start(out=tt_sb[:, :, 0:512], in_=tt_dram[:, :, 0:512])
            nc.sync.dma_start(
                out=tt_sb[:, :, 512:BSLAB], in_=tt_dram[:, :, 512:BSLAB]
            )
            nc.vector.memset(ones_sb, 1.0)
            nc.vector.memset(bias_sb, -CSHIFT)
            nc.vector.memset(warm_sb, 0.0)
            nc.vector.memset(warm2_sb, 1.0)
            # force the exp table-set load (~2.7us) before any sim exists
            nc.scalar.activation(
                out=warm_sb, in_=warm_sb, func=AF.Exp, bias=0.0, scale=1.0
            )
            nc.sync.dma_start(
                out=tt_sb[:, :, BSLAB : 4 * BSLAB],
                in_=tt_dram[:, :, BSLAB : 4 * BSLAB],
            )
            nc.sync.dma_start(
                out=tt_sb[:, :, 4 * BSLAB :], in_=tt_dram[:, :, 4 * BSLAB :]
            )
            # PE power-state warmup: keep the array busy during the DMA
            # wait so the first real matmuls run ramped-up, not cold
            pdummy = psim.tile([PDIM, BSLAB], f32, name="pdummy", tag="sim")
            for _ in range(6):
                nc.tensor.matmul(
                    pdummy[0:1, 0:512],
                    lhsT=ones_sb,
                    rhs=warm2_sb,
                    start=True,
                    stop=True,
                )

            # cq_ps holds all 4 column-sum accumulation streams of one
            # column slab in ONE PSUM bank, at col-group partitions:
            #   row 0: colsum(E) of even 512-half, row 32: odd half,
            #   row 64: colsum(F) even,            row 96: odd.
            cq_tiles = {}
            # (bsl, st, e_sb, f_sb) whose ones-matmuls are deferred one
            # tile so the PE never waits on the just-produced E/F.
            pending = []

            def emit_colsums():
                bsl_, st_, e_, f_ = pending.pop(0)
                cq = cq_tiles[bsl_]
                first = st_ == 0
                last = st_ == NSTRIPE - 1
                for half in range(2):
                    hs = slice(half * 512, (half + 1) * 512)
                    nc.tensor.matmul(
                        cq[half * 32 : half * 32 + 1, :],
                        lhsT=ones_sb,
                        rhs=e_[:, hs],
                        start=first,
                        stop=last,
                        tile_position=(0, half * 32),
                    )
                    nc.tensor.matmul(
                        cq[64 + half * 32 : 64 + half * 32 + 1, :],
                        lhsT=ones_sb,
                        rhs=f_[:, hs],
                        start=first,
                        stop=last,
                        tile_position=(0, 64 + half * 32),
                    )
                if last:
                    drains.append(bsl_)

            def emit_drain():
                bsl_ = drains.pop(0)
                cq = cq_tiles[bsl_]
                scratch = dpool.tile(
                    [PDIM, 512], f32, name=f"dr_{bsl_}", tag="dr"
                )
                # both engines drain disjoint pieces in parallel so their
                # bubbles coincide and the pipeline slips once per drain;
                # the 212/300 split equalizes the two copies at ~470ns
                # ((212+352)/1.2GHz on ScalarE = (300+151)/0.96GHz on DVE)
                nc.scalar.copy(out=scratch[:, 0:212], in_=cq[:, 0:212])
                nc.vector.tensor_copy(scratch[:, 212:512], cq[:, 212:512])
                nc.sync.dma_start(
                    out=cq_dram[:, bsl_, :], in_=scratch[0:PDIM:32, :]
                )

            drains = []
            for bsl in range(NBSLAB):
                cq_tiles[bsl] = pcol.tile(
                    [PDIM, 512], f32, name=f"cq_{bsl}", tag="cq"
                )
                for st in range(NSTRIPE):
                    iss = slice(st * PDIM, (st + 1) * PDIM)
                    idx = bsl * NSTRIPE + st
                    sim_ps = psim.tile(
                        [PDIM, BSLAB], f32, name=f"sim_{bsl}_{st}", tag="sim"
                    )
                    # one DoubleRow matmul per 512-half: K=256 in one shot
                    for half in range(2):
                        hs = slice(half * 512, (half + 1) * 512)
                        jh = slice(
                            bsl * BSLAB + half * 512,
                            bsl * BSLAB + (half + 1) * 512,
                        )
                        nc.tensor.matmul(
                            sim_ps[:, hs],
                            lhsT=it_sb[:, :, iss],
                            rhs=tt_sb[:, :, jh],
                            start=True,
                            stop=True,
                            perf_mode=DR,
                        )
                    e_sb = epool.tile(
                        [PDIM, BSLAB], bf16, name=f"e_{bsl}_{st}", tag="e"
                    )
                    nc.scalar.activation(
                        out=e_sb,
                        in_=sim_ps,
                        func=AF.Exp,
                        bias=bias_sb,
                        scale=1.0 / (TEMP * SIMSCALE),
                        accum_out=racc[:, idx : idx + 1],
                    )
                    f_sb = fpool.tile(
                        [PDIM, BSLAB], bf16, name=f"f_{bsl}_{st}", tag="f"
                    )
                    nc.vector.scalar_tensor_tensor(
                        out=f_sb,
                        in0=sim_ps,
                        scalar=1.0 / SIMSCALE,
                        in1=e_sb,
                        op0=ALU.mult,
                        op1=ALU.mult,
                        accum_out=pacc[:, idx : idx + 1],
                    )
                    pending.append((bsl, st, e_sb, f_sb))
                    if len(pending) > 1:
                        emit_colsums()
                    # drain the previous bslab's cq bank mid-bslab, when
                    # ScalarE has slack
                    if st == 3 and drains:
                        emit_drain()
            while pending:
                emit_colsums()
            # r/p go first: the in-order SP sequencer would otherwise stall
            # on the last drain's semaphore before issuing them
            nc.sync.dma_start(out=r_dram, in_=racc)
            nc.sync.dma_start(out=p_dram, in_=pacc)
            while drains:
                emit_drain()
    nc.compile()
    return nc


def _features_to_kmajor_fp8(feat):
    # [B, D] fp32 -> [128, KCH, B] fp8e4 where [p, c, j] = feat[j, c*128+p]*FSCALE
    import ml_dtypes

    return np.ascontiguousarray(
        (feat.T * FSCALE)
        .reshape(KCH, PDIM, B)
        .transpose(1, 0, 2)
        .astype(ml_dtypes.float8_e4m3)
    )


def kernel(image_features, text_features, b_I, b_T, s_I, s_T, image_ids, text_ids):
    global _prog, last_result
    image_features = np.asarray(image_features, dtype=np.float32)
    text_features = np.asarray(text_features, dtype=np.float32)

    trace = bool(os.environ.get("KERNEL_TRACE"))
    if trace:
        _install_ntff_hook()
    if _prog is None:
        _prog = _build_program()
    from concourse.bass_utils import run_bass_kernel_spmd

    it_full = _features_to_kmajor_fp8(image_features)
    tt_full = _features_to_kmajor_fp8(text_features)
    in_maps = []
    for c in range(NCORES):
        sl = slice(c * SHARD, (c + 1) * SHARD)
        in_maps.append(
            {
                "it_shard": np.ascontiguousarray(it_full[:, :, sl]),
                "tt_full": tt_full,
            }
        )
    last_result = run_bass_kernel_spmd(
        _prog,
        in_maps,
        core_ids=list(range(NCORES)),
        trace=trace,
    )
    res = last_result.results

    # r_out[p, bsl*8+st] partial-sums over bslabs; row for global row
    # (core*1024 + st*128 + p).  Scale: device E' = E * e^-CSHIFT.
    ESC = float(np.exp(CSHIFT))

    def _reduce(out):
        return out.reshape(PDIM, 8, NSTRIPE).sum(axis=1).T.reshape(-1)

    R = np.concatenate(
        [_reduce(r["r_out"].astype(np.float64)) for r in res]
    ) * ESC
    P = np.concatenate(
        [_reduce(r["p_out"].astype(np.float64)) for r in res]
    ) * ESC
    cq = np.sum([r["cq_out"] for r in res], axis=0, dtype=np.float64) * ESC
    C = cq[0:2].transpose(1, 0, 2).reshape(-1)
    Q = cq[2:4].transpose(1, 0, 2).reshape(-1)

    I64 = image_features.astype(np.float64)
    T64 = text_features.astype(np.float64)
    diag = np.einsum("ij,ij->i", I64, T64)
    u = np.exp(-diag / TEMP)

    ids_i = np.asarray(image_ids)
    ids_t = np.asarray(text_ids)
    old_b_I = np.asarray(b_I)[ids_i].astype(np.float64)
    s_old_I = np.asarray(s_I)[ids_i].astype(np.float64)
    old_b_T = np.asarray(b_T)[ids_t].astype(np.float64)
    s_old_T = np.asarray(s_T)[ids_t].astype(np.float64)

    A0 = u * R
    N0 = u * (P - diag * R) / TEMP
    Ki = (1.0 - GAMMA) * s_old_I * np.exp(old_b_I) + GAMMA * A0 / (B - 1)
    image_loss = TEMP * N0 / (Ki + EPS * A0) / (B - 1)

    A0t = u * C
    N0t = u * (Q - diag * C) / TEMP
    Kt = (1.0 - GAMMA) * s_old_T * np.exp(old_b_T) + GAMMA * A0t / (B - 1)
    text_loss = TEMP * N0t / (Kt + EPS * A0t) / (B - 1)

    total = image_loss.mean() + text_loss.mean()
    return np.array(total, dtype=np.float32)

